# revision 38
# baseline (speedup 1.0000x reference)
"""MoE top-2: transposed expert-parallel single-NEFF design, on 8 TRN2 cores.

Orientation: weights are the *stationary* matmul operand (lhsT = 128x128
W blocks), tokens stream as the *moving* operand (rhs = xT columns).  Each
core computes out.T[o, t] = W_e.T @ xT_e for the token-expert pairs it
owns, so cost is exact streamed columns (64 passes per column: 8 o-chunks
x 8 ko) instead of 128-token-padded tiles, and the gate folds into x on
the host ((g*x) @ W = g*(x @ W)) - no per-token scaling on device.

SPMD balance trick: the static program has 2 weight "slots" per core with
uniform chunk widths (slot A: 8x512 cols, slot B: 1 narrow chunk).  Light
experts sit whole in one core's A-bin; heavy experts put their first cA
tokens in an A-bin and spread the overflow over other cores' B-bins.
Which expert sits in which (core, slot) is a host-side input permutation;
the instruction stream is identical on all cores.  Capacities (cA, cB)
are solved at build time from the actual gate nonzero counts.

Chunk 0 runs ko-outer across all 8 PSUM banks (one 256KB W row feeds 8
matmuls = 1.73us of consumption, the slowest order possible, maximizing
slack against the supply rings), with W row 0 landing as 8 contiguous
32KB per-oc blocks; remaining chunks run oc-outer with ko-inner
accumulation.  Measured HW constraints this schedule is built around:
HWDGE rings (sync/scalar dma_start) move ~160GB/s with ~2us startup, the
gpsimd SWDGE ring only ~82GB/s (unused), GPSIMD cannot read PSUM, engine
queues are strict FIFO (so a store waiting on its eviction would convoy
later evictions: all of a chunk's evictions are emitted before its
stores; evictions on vector, stores on the scalar ring early / sync ring
late; the tail chunk splits each eviction across vector+scalar to halve
post-last-matmul latency), and the PE clock-gate (HAM) needs ~3.4us of
continuous matmul activity - dummy warm-up matmuls bridge the first-DMA
wait so the clock is warm when real work starts.  Slot-B evictions pack
into one SBUF tile so its store is a single contiguous DMA.

Host combine: each (core, slot)-bin holds unique tokens of one expert, so
the full output is a plain fancy-index accumulate per bin; bias (if any)
is the rank-1 host add gates @ b.

Self-contained: shapes hardcoded for B=16384, E=8, D=1024, O=1024.
"""

import math
import os
import sys
import types

sys.path.insert(0, "/opt/trn_rl_repo")

import ml_dtypes
import numpy as np

import concourse.bass as bass
import concourse.mybir as mybir
from concourse import bass_utils
from concourse.tile import TileContext

B, E, D, O = 16384, 8, 1024, 1024
N_CORES = 8
P = 128
KO = D // P  # contraction blocks
NOC = O // P  # output partition chunks
OT = 512  # tokens per chunk == one fp32 PSUM bank

_DT_MAP = {
    "float16": (mybir.dt.float16, np.float16),
    "bfloat16": (mybir.dt.bfloat16, ml_dtypes.bfloat16),
    "float32": (mybir.dt.float32, np.float32),
}

MAX_WAITS = int(os.environ.get("MOE_MAX_WAITS", "1"))


def _patch_tile_drain():
    """Public-walrus workaround: walrus codegen rejects instructions carrying
    more than a couple of sync-wait commands.  Tile's add_semaphores can put
    several waits on one instruction (and the kernel-tail drain carries one
    per live processor).  Hoist excess waits onto single-wait nop carriers
    emitted just before the instruction on the same engine."""
    from concourse.tile import TileContext as TC
    from concourse.vector_clock import ScopedClock

    if getattr(TC, "_moe_drain_patched", False):
        return

    orig_add = TC._add_instruction

    def _add_instruction(self, inst):
        si = getattr(inst, "sync_info", None)
        waits = list(si.on_wait or []) if si is not None else []
        if len(waits) > MAX_WAITS:
            hoist = waits[: len(waits) - MAX_WAITS]
            keep = waits[len(waits) - MAX_WAITS :]
            for w in hoist:
                nop = mybir.InstNoOp(
                    name=self.nc.get_next_instruction_name(),
                    engine=inst.engine,
                    bass_nofuse=True,
                    sync_info=mybir.SyncInfo(on_wait=[w], on_update=[]),
                )
                orig_add(self, nop)
            inst.sync_info = mybir.SyncInfo(
                on_wait=keep, on_update=list(si.on_update or [])
            )
        orig_add(self, inst)

    def _drain_and_barrier(self, tick_clock, wait_clock):
        carrier = self.nc.sync.nop(nofuse=True)
        wait_clock.add_sem_waits(
            carrier.ins, ScopedClock({None: tick_clock.global_clock})
        )
        si = carrier.ins.sync_info
        waits = list(si.on_wait or []) if si is not None else []
        if len(waits) > 1:
            carrier.ins.sync_info = mybir.SyncInfo(
                on_wait=waits[:1], on_update=list(si.on_update or [])
            )
            for w in waits[1:]:
                extra = self.nc.sync.nop(nofuse=True)
                extra.ins.sync_info = mybir.SyncInfo(on_wait=[w], on_update=[])
        self.nc.sync.drain()
        self.nc.all_engine_barrier()
        assert self.sems is not None
        popped = self.nc._tile_sem_poison_stack.pop()
        assert popped is self._sem_poison
        self.nc.clear_and_free_semaphores(list(self.sems.allocated().values()))
        self.nc.all_engine_barrier()

    TC._add_instruction = _add_instruction
    TC._drain_and_barrier = _drain_and_barrier
    TC._moe_drain_patched = True


def _plan(ne):
    """Choose slot capacities and per-core bin assignment.

    Returns (cA, cB, plans): cA is a multiple of 512 (slot A capacity), cB
    the slot-B chunk width (0 = no B slot), plans[c] = dict with keys
    eA, tA0 (token offset into expert eA's list), nA, eB, tB0, nB."""
    total = sum(ne)
    cA = 512 * max(1, math.ceil(total / (N_CORES * 512)))
    while True:
        over = [max(0, n - cA) for n in ne]
        if not any(over):
            cB = 0
            break
        cB = None
        for cand in range(16, OT + 1, 16):
            if sum(math.ceil(o / cand) for o in over if o) <= N_CORES:
                cB = cand
                break
        if cB is not None:
            break
        cA += 512
    plans = [
        {"eA": e, "tA0": 0, "nA": min(ne[e], cA), "eB": None, "tB0": 0, "nB": 0}
        for e in range(N_CORES)
    ]
    core = 0
    for e in range(len(ne)):
        off = cA
        while off < ne[e]:
            take = min(cB, ne[e] - off)
            while plans[core]["eB"] is not None:
                core += 1
            plans[core].update(eB=e, tB0=off, nB=take)
            off += take
    return cA, cB, plans


def _build_program(nA_chunks, cB, dt, ydt):
    """Single NEFF shared by all cores.  Chunk schedule: nA_chunks x 512-col
    slot-A chunks + (if cB) one cB-col slot-B chunk."""
    has_b = cB > 0
    C = nA_chunks + (1 if has_b else 0)
    CW = C * OT
    nc = bass.Bass(target_bir_lowering=False, trn_type="TRN2")
    xg_d = nc.dram_tensor("xg", [C, KO, P, OT], dt, kind="ExternalInput")
    wa_d = nc.dram_tensor("wa", [KO, P, O], dt, kind="ExternalInput")
    # W row 0 again, as contiguous per-oc 32KB blocks: the first 8 matmuls
    # each wait on one small block instead of a lumpy multi-piece row.
    wa0_d = nc.dram_tensor("wa0", [NOC, P, P], dt, kind="ExternalInput")
    if has_b:
        wb_d = nc.dram_tensor("wb", [KO, P, O], dt, kind="ExternalInput")
    ya_d = nc.dram_tensor("ya", [nA_chunks, NOC, P, OT], ydt, kind="ExternalOutput")
    if has_b:
        yb_d = nc.dram_tensor("yb", [P, NOC * cB], ydt, kind="ExternalOutput")

    with TileContext(nc) as tc:
        with (
            tc.tile_pool(name="wp", bufs=1) as wpool,
            tc.tile_pool(name="xp", bufs=1) as xpool,
            tc.tile_pool(name="yt", bufs=14) as ypool,
            tc.tile_pool(name="ybp", bufs=1) as ybpool,
            tc.tile_pool(name="ps", bufs=8, space="PSUM") as pspool,
        ):
            wa_t = wpool.tile([P, KO, O], dt, tag="wa")
            x_t = xpool.tile([P, KO, CW], dt, tag="x")
            warm_t = wpool.tile([P, P + P], dt, tag="warm")
            if has_b:
                wb_t = wpool.tile([P, KO, O], dt, tag="wb")
                yb_t = ybpool.tile([P, NOC * cB], ydt, tag="yb")

            # Ring budget (HWDGE ~160GB/s, SWDGE ~80GB/s, ~2.5us startup):
            # scalar ring: slot-A weights then all output stores; sync ring:
            # all x chunks then slot-B weights; no SWDGE DMAs.  The first W
            # row and first x slice go in small pieces so the first real
            # matmul's waits cover ~32-64KB, not 128-256KB.
            # W row 1 rides the sync ring between the first two x slices:
            # the scalar ring then supplies rows 2..7 with 2x slack instead
            # of racing chunk-0 consumption at 92% load (which stalled the
            # PE ~4-6us on jittery cores).
            for oc in range(NOC):
                nc.scalar.dma_start(
                    out=wa_t[:, 0, oc * P : (oc + 1) * P], in_=wa0_d[oc]
                )
            for ko in range(2, KO):
                nc.scalar.dma_start(out=wa_t[:, ko, :], in_=wa_d[ko])
            for c in range(C):
                for ko in range(KO):
                    nc.sync.dma_start(
                        out=x_t[:, ko, c * OT : (c + 1) * OT], in_=xg_d[c, ko]
                    )
                    if c == 0 and ko == 0:
                        nc.sync.dma_start(out=wa_t[:, 1, :], in_=wa_d[1])
            # Slot-B weights queue behind the x stream; needed only near the
            # end (land ~80us in, consumed ~115us in).
            if has_b:
                for ko in range(KO):
                    nc.sync.dma_start(out=wb_t[:, ko, :], in_=wb_d[ko])

            # HAM warm-up: dummy matmuls fill the first-DMA wait window
            # (~7.9-9.5us) with continuous PE activity so the clock-gate
            # lifts ~3.4us after they START, not after the first real matmul.
            # 24 cold 128-wide matmuls (~107ns each) span ~2.6us.
            nc.vector.memset(warm_t[:], 0.25)
            warm_ps = pspool.tile([P, OT], mybir.dt.float32, tag="ps")
            for _ in range(24):
                nc.tensor.matmul(
                    out=warm_ps[:, :P],
                    lhsT=warm_t[:, :P],
                    rhs=warm_t[:, P:],
                    start=True,
                    stop=True,
                )

            def evict(i, out_ap, in_ap):
                # All evictions on vector, all stores on scalar: each engine's
                # strict FIFO then has no cross-dependency convoys.
                nc.vector.tensor_scalar_mul(out=out_ap, in0=in_ap, scalar1=1.0)

            # Chunk 0: ko-outer across all 8 PSUM banks, so the matmul for
            # (ko, oc) needs only W row ko / x slice ko - both land early.
            # Chunk 0 runs ko-outer across all 8 PSUM banks: one W row feeds
            # 8 matmuls (1.73us of consumption per 256KB row), the slowest
            # consumption order possible, maximizing slack against the
            # supply rings.  (warm_ps's bank is recycled as ps0_7 - legal
            # because every warm matmul is emitted before ps0_7's first use.)
            ps0 = [
                pspool.tile([P, OT], mybir.dt.float32, tag="ps", name=f"ps0_{i}")
                for i in range(NOC)
            ]
            for ko in range(KO):
                for oc in range(NOC):
                    nc.tensor.matmul(
                        out=ps0[oc][:],
                        lhsT=wa_t[:, ko, oc * P : (oc + 1) * P],
                        rhs=x_t[:, ko, 0:OT],
                        start=(ko == 0),
                        stop=(ko == KO - 1),
                    )
            # Evictions before stores per chunk: a store dma_start on the
            # scalar ring waits for its eviction and would head-of-line block
            # later evictions queued behind it on the strict-FIFO engine.
            ys0 = []
            for oc in range(NOC):
                y = ypool.tile([P, OT], ydt, tag="y", name=f"y0_{oc}")
                evict(oc, y[:], ps0[oc][:])
                ys0.append(y)
            for oc in range(NOC):
                nc.scalar.dma_start(out=ya_d[0, oc], in_=ys0[oc][:])

            def a_chunk(c, last=False):
                # Late chunks store on the sync ring (idle once x and W-B
                # have streamed) to halve the store-queue load per ring.
                store_eng = nc.scalar if c < nA_chunks // 2 else nc.sync
                ys = []
                for oc in range(NOC):
                    ps = pspool.tile([P, OT], mybir.dt.float32, tag="ps", name="ps")
                    for ko in range(KO):
                        nc.tensor.matmul(
                            out=ps[:],
                            lhsT=wa_t[:, ko, oc * P : (oc + 1) * P],
                            rhs=x_t[:, ko, c * OT : (c + 1) * OT],
                            start=(ko == 0),
                            stop=(ko == KO - 1),
                        )
                    y = ypool.tile([P, OT], ydt, tag="y", name=f"y{c}_{oc}")
                    if last:
                        # Tail chunk: halve post-last-matmul eviction latency
                        # by splitting each eviction across vector + scalar
                        # (stores go on sync here, so scalar's FIFO is free).
                        nc.vector.tensor_scalar_mul(
                            out=y[:, : OT // 2], in0=ps[:, : OT // 2], scalar1=1.0
                        )
                        nc.scalar.copy(out=y[:, OT // 2 :], in_=ps[:, OT // 2 :])
                    else:
                        evict(oc, y[:], ps[:])
                    ys.append(y)
                for oc in range(NOC):
                    store_eng.dma_start(out=ya_d[c, oc], in_=ys[oc][:])

            def b_chunk():
                # Narrow slot-B chunk; evictions pack into one tile so the
                # store is a single contiguous DMA.
                base = nA_chunks * OT
                for oc in range(NOC):
                    ps = pspool.tile([P, OT], mybir.dt.float32, tag="ps", name="ps")
                    for ko in range(KO):
                        nc.tensor.matmul(
                            out=ps[:, :cB],
                            lhsT=wb_t[:, ko, oc * P : (oc + 1) * P],
                            rhs=x_t[:, ko, base : base + cB],
                            start=(ko == 0),
                            stop=(ko == KO - 1),
                        )
                    evict(oc, yb_t[:, oc * cB : (oc + 1) * cB], ps[:, :cB])
                nc.scalar.dma_start(out=yb_d[:], in_=yb_t[:])

            # Remaining slot-A chunks, with the B chunk second-to-last: the
            # kernel then ends on an A chunk whose evictions/stores pipeline
            # per-oc, instead of on the B chunk's packed store.
            for c in range(1, nA_chunks - 1):
                a_chunk(c)
            if has_b:
                b_chunk()
            if nA_chunks > 1:
                a_chunk(nA_chunks - 1, last=True)
    return nc


def kernel(x, gates, W, b):
    _patch_tile_drain()
    dt_name = os.environ.get("MOE_DT", "float16")
    ydt_name = os.environ.get("MOE_YDT", "float16")
    dt, np_dt = _DT_MAP[dt_name]
    ydt, _ = _DT_MAP[ydt_name]

    x = np.ascontiguousarray(np.asarray(x, np.float32))
    gates = np.asarray(gates, np.float32)
    W = np.asarray(W, np.float32)
    b = np.asarray(b, np.float32)

    toks = [np.nonzero(gates[:, e] > 0)[0] for e in range(E)]
    ne = [len(t) for t in toks]
    cA, cB, plans = _plan(ne)
    nA_chunks = cA // OT
    has_b = cB > 0
    C = nA_chunks + (1 if has_b else 0)

    def build_slot(e, t0, n, cap):
        """[KO, 128, cap] transposed gate-scaled x for one bin, fp16."""
        ids = toks[e][t0 : t0 + n]
        buf = np.zeros((cap, D), np.float32)
        buf[:n] = x[ids] * gates[ids, e][:, None]
        return np.ascontiguousarray(buf.T).astype(np_dt).reshape(KO, P, cap)

    in_maps = []
    for c in range(N_CORES):
        pl = plans[c]
        xg = np.zeros((C, KO, P, OT), np_dt)
        xa = build_slot(pl["eA"], pl["tA0"], pl["nA"], cA)
        xg[:nA_chunks] = xa.reshape(KO, P, nA_chunks, OT).transpose(2, 0, 1, 3)
        wa_np = W[pl["eA"]].astype(np_dt)
        m = {
            "xg": xg,
            "wa": wa_np.reshape(KO, P, O),
            "wa0": np.ascontiguousarray(
                wa_np[:P].reshape(P, NOC, P).transpose(1, 0, 2)
            ),
        }
        if has_b:
            eB = pl["eB"] if pl["eB"] is not None else pl["eA"]
            xg[nA_chunks, :, :, :cB] = build_slot(eB, pl["tB0"], pl["nB"], cB)
            m["wb"] = W[eB].astype(np_dt).reshape(KO, P, O)
        in_maps.append(m)

    nc = _build_program(nA_chunks, cB, dt, ydt)

    trace = os.environ.get("MOE_TRACE", "0") == "1"
    kwargs = {}
    if trace:
        _install_ntff_shim()
        kwargs = dict(trace=True, trace_cores=list(range(N_CORES)))

    res = bass_utils.run_bass_kernel_spmd(
        nc, in_maps, core_ids=list(range(N_CORES)), **kwargs
    )
    if trace and res.exec_time_ns is not None:
        print(f"HW exec time: {res.exec_time_ns} ns "
              f"(mean {res.mean_exec_time_ns:.0f}; cA {cA} cB {cB})")

    out = np.zeros((B, O), np.float32)
    for c in range(N_CORES):
        pl = plans[c]
        ya = res.results[c]["ya"]  # [nA_chunks, NOC, P, OT] (c, oc, p, w)
        arr = ya.transpose(0, 3, 1, 2).reshape(cA, O).astype(np.float32)
        ids = toks[pl["eA"]][pl["tA0"] : pl["tA0"] + pl["nA"]]
        out[ids] += arr[: pl["nA"]]
        if has_b and pl["eB"] is not None:
            yb = res.results[c]["yb"]  # [P, NOC*cB] (p, oc*cB + w)
            arrb = yb.reshape(P, NOC, cB).transpose(2, 1, 0).reshape(cB, O)
            ids = toks[pl["eB"]][pl["tB0"] : pl["tB0"] + pl["nB"]]
            out[ids] += arrb[: pl["nB"]].astype(np.float32)
    if np.any(b != 0):
        out += gates @ b
    return out


def _install_ntff_shim():
    """Best-effort: register the missing antenv.axon_hooks NTFF profile hook
    so trace=True yields exec_time_ns.  Only used when MOE_TRACE=1."""
    try:
        import antenv
        from trn_agent_boot.trn_boot import _ntff_profile_via_ctypes

        if "antenv.axon_hooks" in sys.modules:
            return
        hooks = types.ModuleType("antenv.axon_hooks")
        hook = _ntff_profile_via_ctypes("/opt/axon/libaxon_pjrt.so")
        hooks.get_axon_ntff_profile_hook = lambda: hook
        hooks.set_axon_ntff_profile_hook = lambda h: None
        sys.modules["antenv.axon_hooks"] = hooks
        antenv.axon_hooks = hooks
        bass_utils.upload_artifacts = lambda tmpdir: tmpdir
    except Exception as e:  # pragma: no cover
        print(f"ntff shim unavailable: {e}", file=sys.stderr)
